# revision 5
# baseline (speedup 1.0000x reference)
"""GCN (2-layer) + edge-dot decode on 8 TRN2 NeuronCores.

Math (per GCN layer, with dinv = rsqrt(indeg+1)):
    out[v] = dinv[v] * ( sum_{e: dst=v} hs[src_e] + hs[v] ) + b,  hs = dinv (.) (x @ W)
so no per-edge norm values are needed anywhere; all scaling is per-node.

Layer 2 is commuted past W2:  z = (dinv (.) (A_hat g)) @ W2 + b2,  g = dinv (.) relu(out1).

Device layout: nodes permuted (degree-sorted, core-striped so each core owns a
contiguous 6272-row slice), aggregation via dma_gather row-gathers + TensorE
identity-matmul accumulation into PSUM. hs/g/z tables are AllGathered between
phases. Indices for dma_gather are int16, so tables are addressed via two
overlapping frames (A: rows 0.., B: rows 32767..); permutation places a
zero (pad) node at id 32766 and at the tail so both frames have a zero pad row.
"""

import sys
import numpy as np
from contextlib import ExitStack

sys.path.insert(0, "/opt/trn_rl_repo")

import concourse.bass as bass
import concourse.mybir as mybir
from concourse.bass_utils import run_bass_kernel_spmd
from concourse.tile import TileContext, add_dep_helper
from concourse.masks import make_identity
from concourse.library_config import mlp
from concourse.library_overlay import lower_extended_insts

N, E, L = 50000, 800000, 200000
IN, HID, OUT = 256, 128, 64
C = 8                      # cores
NP = 50176                 # padded node count = 392 blocks of 128
NPC = NP // C              # 6272 nodes per core
BPC = NPC // 128           # 49 blocks per core
ASPLIT = 32766             # pad node id; A frame serves ids <= 32765
BBASE = 32767              # B frame base row; B idx = id - 32767
APAD = 32766               # A-frame pad index (zero row)
BPAD = NP - 1 - BBASE      # B-frame pad index -> id 50175 (zero row)
DEC_CHUNK = 16             # decode gather chunk: 16*128 idxs


CUSTOM_ISA_OPCODES = {"DMAGatherAnt", "DMAScatterAddAnt"}


def _fix_sync_waits(nc):
    """This container's walrus accepts at most one sync-wait per instruction
    and none on custom ISA ucode ops; hoist extras onto preceding drains."""
    f = nc.m.functions[0]
    for b in f.blocks:
        insts = b.instructions
        i = 0
        while i < len(insts):
            ins = insts[i]
            si = ins.sync_info
            nw = len(si.on_wait) if (si is not None and si.on_wait is not None) else 0
            keep = 0 if str(ins.opcode) in CUSTOM_ISA_OPCODES else 1
            if nw > keep:
                waits = list(si.on_wait)
                hoist, keepw = waits[: nw - keep], waits[nw - keep:]
                for j, w in enumerate(hoist):
                    d = mybir.InstDrain(name=f"{ins.name}-wsplit{j}")
                    d.engine = ins.engine
                    d.sync_info = mybir.SyncInfo(on_wait=[w], on_update=[])
                    insts.insert(i + j, d)
                si.on_wait = keepw
                i += len(hoist)
            i += 1


def _sortedpos(p):
    """final position -> position in the degree-sorted sequence."""
    core = p // NPC
    k = (p % NPC) // 128
    lane = p % 128
    return 128 * (8 * k + core) + lane


def _wrap_idx(flat):
    """[n] int16 -> [128, n//16] wrapped in 16 partitions, replicated x8."""
    n = flat.shape[0]
    arr = np.empty((16, n // 16), dtype=np.int16)
    arr[:, :] = flat.reshape(n // 16, 16).T
    return np.tile(arr, (8, 1))


def _prepare(edge_index, edge_label_index):
    src = np.asarray(edge_index[0], dtype=np.int64)
    dst = np.asarray(edge_index[1], dtype=np.int64)
    la = np.asarray(edge_label_index[0], dtype=np.int64)
    lb = np.asarray(edge_label_index[1], dtype=np.int64)

    deg = np.bincount(dst, minlength=N).astype(np.int64)

    # ---- permutation: degree-sorted, core-striped; pads at 32766 + tail ----
    sorted_real = np.argsort(-deg, kind="stable")
    PAD = -1
    fp = _sortedpos(ASPLIT)  # sorted position that maps to final id 32766
    seq = np.empty(NP, dtype=np.int64)
    seq[:fp] = sorted_real[:fp]
    seq[fp] = PAD
    seq[fp + 1: 50001] = sorted_real[fp:]
    seq[50001:] = PAD
    sp = _sortedpos(np.arange(NP))
    final_perm = seq[sp]                      # final position -> orig node (-1 pad)
    invpos = np.full(N, -1, dtype=np.int64)
    real_mask = final_perm >= 0
    invpos[final_perm[real_mask]] = np.nonzero(real_mask)[0]
    assert final_perm[ASPLIT] == PAD and final_perm[NP - 1] == PAD

    ps = invpos[src]
    pd = invpos[dst]
    assert ps.min() >= 0 and pd.min() >= 0

    # ---- per-(block,half) round tables ----
    half = (ps > ASPLIT).astype(np.int64)     # 0 = A (ps<=32765), 1 = B (ps>=32767)
    order = np.lexsort((ps, half, pd))
    pd_s, half_s, ps_s = pd[order], half[order], ps[order]
    key = pd_s * 2 + half_s
    starts = np.zeros(E, dtype=np.int64)
    newgrp = np.empty(E, dtype=bool)
    newgrp[0] = True
    newgrp[1:] = key[1:] != key[:-1]
    gidx = np.nonzero(newgrp)[0]
    grp = np.cumsum(newgrp) - 1
    rank = np.arange(E) - gidx[grp]

    blk = pd_s // 128
    lane = pd_s % 128
    core = pd_s // NPC
    slot = (pd_s % NPC) // 128

    nblocks = NP // 128
    KA = np.zeros(nblocks, dtype=np.int64)
    KB = np.zeros(nblocks, dtype=np.int64)
    np.maximum.at(KA, blk[half_s == 0], rank[half_s == 0] + 1)
    np.maximum.at(KB, blk[half_s == 1], rank[half_s == 1] + 1)
    KA2 = KA.reshape(nblocks // 8, 8)  # [slot-ordered? no: sorted block j = 8k+c]
    # block(core c, slot k) has final block index c*BPC + k ... careful:
    # final block index b = p//128 with p = c*NPC + k*128 + lane -> b = c*BPC + k
    KAhat = np.zeros(BPC, dtype=np.int64)
    KBhat = np.zeros(BPC, dtype=np.int64)
    for k in range(BPC):
        KAhat[k] = int(KA[[c * BPC + k for c in range(C)]].max())
        KBhat[k] = int(KB[[c * BPC + k for c in range(C)]].max())

    offA = np.zeros(BPC + 1, dtype=np.int64)
    offB = np.zeros(BPC + 1, dtype=np.int64)
    offA[1:] = np.cumsum(KAhat)
    offB[1:] = np.cumsum(KBhat)

    idxA = np.full((C, offA[-1], 128), APAD, dtype=np.int16)
    idxB = np.full((C, offB[-1], 128), BPAD, dtype=np.int16)
    mA = half_s == 0
    idxA[core[mA], offA[slot[mA]] + rank[mA], lane[mA]] = ps_s[mA].astype(np.int16)
    mB = half_s == 1
    idxB[core[mB], offB[slot[mB]] + rank[mB], lane[mB]] = (ps_s[mB] - BBASE).astype(np.int16)

    # ---- degp per core [128, BPC]: deg+1 (f32), 1e30 for pads ----
    degp = np.full(NP, 1e30, dtype=np.float32)
    degp[real_mask] = (deg[final_perm[real_mask]] + 1).astype(np.float32)
    degp_core = degp.reshape(C, BPC, 128).transpose(0, 2, 1).copy()  # [C,128,BPC]

    # ---- decode tables ----
    pa = invpos[la]
    pb = invpos[lb]
    LPC = L // C
    grp4 = (pa > ASPLIT) * 2 + (pb > ASPLIT)
    dec_sortidx = []      # per core: label ids (global) in gather order incl pads(-1)
    cnt = np.zeros((C, 4), dtype=np.int64)
    for c in range(C):
        gg = grp4[c * LPC:(c + 1) * LPC]
        for g in range(4):
            cnt[c, g] = int((gg == g).sum())
    LG = ((cnt.max(axis=0) + 127) // 128) * 128          # compiled group sizes
    # round each group up to DEC_CHUNK*128 multiples? chunks can straddle: we
    # chunk each group independently; group sizes already /128.
    decA = []  # per core flat int16 idx for endpoint a, per group concat
    decB = []
    for c in range(C):
        gg = grp4[c * LPC:(c + 1) * LPC]
        ia = np.full(int(LG.sum()), -1, dtype=np.int64)   # global label id or -1
        fa = np.empty(int(LG.sum()), dtype=np.int16)
        fb = np.empty(int(LG.sum()), dtype=np.int16)
        off = 0
        for g in range(4):
            ids = np.nonzero(gg == g)[0] + c * LPC
            n = len(ids)
            ia[off:off + n] = ids
            ha, hb = g // 2, g % 2
            a_idx = pa[ids] - (BBASE if ha else 0)
            b_idx = pb[ids] - (BBASE if hb else 0)
            fa[off:off + n] = a_idx.astype(np.int16)
            fb[off:off + n] = b_idx.astype(np.int16)
            fa[off + n: off + LG[g]] = BPAD if ha else APAD
            fb[off + n: off + LG[g]] = BPAD if hb else APAD
            off += LG[g]
        dec_sortidx.append(ia)
        decA.append(fa)
        decB.append(fb)

    # ---- flatten gather idx streams into the wrapped int16 SBUF layout ----
    # order: per block k: [A gather], [B gather]; then decode chunks:
    # per group g: chunks of a; chunks of b (chunk size DEC_CHUNK*128).
    gathers = []   # (kind, num_idxs, table, rounds) schedule, same all cores
    for k in range(BPC):
        gathers.append(("LA", int(KAhat[k]) * 128, k))
        gathers.append(("LB", int(KBhat[k]) * 128, k))
    dec_chunks = []
    for g in range(4):
        ha, hb = g // 2, g % 2
        ng = int(LG[g]) // 128
        for s in range(0, ng, DEC_CHUNK):
            ch = min(DEC_CHUNK, ng - s)
            dec_chunks.append((g, s, ch, ha, hb))

    idx16 = []
    for c in range(C):
        parts = []
        for k in range(BPC):
            parts.append(_wrap_idx(idxA[c, offA[k]:offA[k + 1]].reshape(-1)))
            parts.append(_wrap_idx(idxB[c, offB[k]:offB[k + 1]].reshape(-1)))
        goff = np.concatenate(([0], np.cumsum(LG)))
        for (g, s, ch, ha, hb) in dec_chunks:
            base = int(goff[g]) + s * 128
            parts.append(_wrap_idx(decA[c][base: base + ch * 128]))
        for (g, s, ch, ha, hb) in dec_chunks:
            base = int(goff[g]) + s * 128
            parts.append(_wrap_idx(decB[c][base: base + ch * 128]))
        idx16.append(np.ascontiguousarray(np.concatenate(parts, axis=1)))

    return dict(
        final_perm=final_perm, invpos=invpos, real_mask=real_mask,
        KAhat=KAhat, KBhat=KBhat, offA=offA, offB=offB,
        degp_core=degp_core, idx16=idx16,
        dec_chunks=dec_chunks, LG=LG, dec_sortidx=dec_sortidx,
    )


def _build(prep):
    KAhat, KBhat = prep["KAhat"], prep["KBhat"]
    dec_chunks = prep["dec_chunks"]
    TOTW = prep["idx16"][0].shape[1]
    ndec_cols = sum(ch for (_, _, ch, _, _) in dec_chunks)

    nc = bass.Bass(num_devices=C)
    f32 = mybir.dt.float32
    xT_d = nc.dram_tensor("xT", [IN, NPC], f32, kind="ExternalInput")
    W1_d = nc.dram_tensor("W1", [IN, HID], f32, kind="ExternalInput")
    b1_d = nc.dram_tensor("b1", [1, HID], f32, kind="ExternalInput")
    W2_d = nc.dram_tensor("W2", [HID, OUT], f32, kind="ExternalInput")
    b2_d = nc.dram_tensor("b2", [1, OUT], f32, kind="ExternalInput")
    degp_d = nc.dram_tensor("degp", [128, BPC], f32, kind="ExternalInput")
    idx_d = nc.dram_tensor("idx16", [128, TOTW], mybir.dt.int16, kind="ExternalInput")
    out_d = nc.dram_tensor("out", [128, ndec_cols], f32, kind="ExternalOutput")

    ag1_in = nc.dram_tensor("ag1_in", [NPC, HID], f32)
    tab1 = nc.dram_tensor("tab1", [NP, HID], f32)
    ag2_in = nc.dram_tensor("ag2_in", [NPC, HID], f32)
    tab2 = nc.dram_tensor("tab2", [NP, HID], f32)
    ag3_in = nc.dram_tensor("ag3_in", [NPC, OUT], f32)
    ztab = nc.dram_tensor("ztab", [NP, OUT], f32)

    with TileContext(nc) as tc, ExitStack() as ctx:
        const = ctx.enter_context(tc.tile_pool(name="const", bufs=1))
        own = ctx.enter_context(tc.tile_pool(name="own", bufs=1))
        lp = ctx.enter_context(tc.tile_pool(name="lhsT", bufs=4))
        gp = ctx.enter_context(tc.tile_pool(name="gath", bufs=3))
        dgp = ctx.enter_context(tc.tile_pool(name="dgath", bufs=4))
        pp = ctx.enter_context(tc.tile_pool(name="psA", bufs=2, space="PSUM"))
        pz = ctx.enter_context(tc.tile_pool(name="psZ", bufs=2, space="PSUM"))
        sp_ = ctx.enter_context(tc.tile_pool(name="stage", bufs=4))

        ll = nc.gpsimd.load_library(mlp)

        ident = const.tile([128, 128], f32)
        make_identity(nc, ident[:])

        idx_sb = const.tile([128, TOTW], mybir.dt.int16)
        idma = nc.sync.dma_start(out=idx_sb[:], in_=idx_d[:, :])
        add_dep_helper(idma.ins, ll.ins, reason="idx after lib load")

        # num_idxs registers, one per distinct value
        kvals = sorted(({int(v) * 128 for v in KAhat} | {int(v) * 128 for v in KBhat}
                       | {ch * 128 for (_, _, ch, _, _) in dec_chunks}) - {0})
        kreg = {}
        for v in kvals:
            r = ctx.enter_context(nc.gpsimd.register(f"nidx{v}"))
            nc.gpsimd.reg_mov(r, v)
            kreg[v] = r

        # weights / bias / dinv
        W1_sb = []
        for i in range(2):
            w1t = const.tile([128, HID], f32, tag=f"w1_{i}", name=f"w1_{i}")
            W1_sb.append(w1t)
        for i in range(2):
            nc.sync.dma_start(out=W1_sb[i][:], in_=W1_d[i * 128:(i + 1) * 128, :])
        W2_sb = const.tile([128, OUT], f32)
        nc.sync.dma_start(out=W2_sb[:], in_=W2_d[:, :])

        ones_row = const.tile([1, 128], f32)
        nc.vector.memset(ones_row[:], 1.0)
        b1_row = const.tile([1, HID], f32)
        nc.sync.dma_start(out=b1_row[:], in_=b1_d[:, :])
        b2_row = const.tile([1, OUT], f32)
        nc.sync.dma_start(out=b2_row[:], in_=b2_d[:, :])
        bias1 = const.tile([128, HID], f32)
        bps = pz.tile([128, HID], f32, tag="qt")
        nc.tensor.matmul(out=bps[:], lhsT=ones_row[:], rhs=b1_row[:], start=True, stop=True)
        nc.scalar.activation(out=bias1[:], in_=bps[:], func=mybir.ActivationFunctionType.Copy)
        bias2 = const.tile([128, OUT], f32)
        bps2 = pz.tile([128, OUT], f32, tag="qt")
        nc.tensor.matmul(out=bps2[:], lhsT=ones_row[:], rhs=b2_row[:], start=True, stop=True)
        nc.scalar.activation(out=bias2[:], in_=bps2[:], func=mybir.ActivationFunctionType.Copy)

        degp_sb = const.tile([128, BPC], f32)
        nc.sync.dma_start(out=degp_sb[:], in_=degp_d[:, :])
        rec = const.tile([128, BPC], f32)
        nc.vector.reciprocal(out=rec[:], in_=degp_sb[:])
        dinv = const.tile([128, BPC], f32)
        nc.scalar.activation(out=dinv[:], in_=rec[:], func=mybir.ActivationFunctionType.Sqrt)

        hs_own = own.tile([128, NPC], f32)   # 49 blocks of layer-1 hs (this core)
        g_own = own.tile([128, NPC], f32)    # 49 blocks of layer-2 input g

        # ---------------- Phase A: GEMM1 -> hs1 ----------------
        wr1 = []
        for k in range(BPC):
            ps = pp.tile([128, HID], f32, tag="main")
            for i in range(2):
                lt = lp.tile([128, 128], f32, tag="lhsT")
                nc.sync.dma_start(out=lt[:], in_=xT_d[i * 128:(i + 1) * 128, k * 128:(k + 1) * 128])
                nc.tensor.matmul(out=ps[:], lhsT=lt[:], rhs=W1_sb[i][:],
                                 start=(i == 0), stop=(i == 1))
            nc.scalar.activation(out=hs_own[:, k * 128:(k + 1) * 128], in_=ps[:],
                                 func=mybir.ActivationFunctionType.Copy,
                                 scale=dinv[:, k:k + 1])
            wr1.append(nc.sync.dma_start(out=ag1_in[k * 128:(k + 1) * 128, :],
                                         in_=hs_own[:, k * 128:(k + 1) * 128]))

        cc1 = nc.gpsimd.collective_compute(
            "AllGather", mybir.AluOpType.bypass,
            replica_groups=[list(range(C))],
            ins=[ag1_in.ap().opt()], outs=[tab1.ap().opt()],
        )
        for w in wr1:
            add_dep_helper(cc1.ins, w.ins, reason="AG1 after hs writes")

        offA, offB = prep["offA"], prep["offB"]

        def aggregate(k, tab, own_tile, cc, psum_tag):
            """psum = sum_rounds gathered + self(own block)."""
            ps = pp.tile([128, HID], f32, tag="main")
            first = [True]

            def mm(rhs_ap, stop=False):
                nc.tensor.matmul(out=ps[:], lhsT=ident[:], rhs=rhs_ap,
                                 start=first[0], stop=stop)
                first[0] = False

            for (tag, off, khat, base) in (
                ("A", offA[k], int(prep["KAhat"][k]), 0),
                ("B", offB[k], int(prep["KBhat"][k]), BBASE),
            ):
                if khat == 0:
                    continue
                # idx cols: A gathers for blocks laid first per (k): layout is
                # per k: A then B. col offset:
                coff = (offA[k] + offB[k]) * 8
                if tag == "B":
                    coff = (offA[k + 1] + offB[k]) * 8
                w = khat * 8
                gt = gp.tile([128, khat, HID], f32, tag="gt")
                gi = nc.gpsimd.dma_gather(
                    gt[:], tab[base:, :], idx_sb[:, coff:coff + w],
                    khat * 128, kreg[khat * 128], HID, single_packet=False)
                add_dep_helper(gi.ins, cc.ins, reason="gather after AG")
                for r in range(khat):
                    mm(gt[:, r, :])
            mm(own_tile[:, k * 128:(k + 1) * 128], stop=True)
            return ps

        # ---------------- Phase B: layer-1 aggregation -> g ----------------
        wr2 = []
        for k in range(BPC):
            ps = aggregate(k, tab1, hs_own, cc1, "agg1")
            t1 = sp_.tile([128, HID], f32, tag="t1")
            nc.scalar.activation(out=t1[:], in_=ps[:],
                                 func=mybir.ActivationFunctionType.Copy,
                                 scale=dinv[:, k:k + 1])
            t2 = sp_.tile([128, HID], f32, tag="t2")
            nc.vector.tensor_add(out=t2[:], in0=t1[:], in1=bias1[:])
            nc.scalar.activation(out=g_own[:, k * 128:(k + 1) * 128], in_=t2[:],
                                 func=mybir.ActivationFunctionType.Relu,
                                 scale=dinv[:, k:k + 1])
            wr2.append(nc.sync.dma_start(out=ag2_in[k * 128:(k + 1) * 128, :],
                                         in_=g_own[:, k * 128:(k + 1) * 128]))

        cc2 = nc.gpsimd.collective_compute(
            "AllGather", mybir.AluOpType.bypass,
            replica_groups=[list(range(C))],
            ins=[ag2_in.ap().opt()], outs=[tab2.ap().opt()],
        )
        for w in wr2:
            add_dep_helper(cc2.ins, w.ins, reason="AG2 after g writes")

        # ---------------- Phase C: layer-2 aggregation + GEMM2 -> z ----------------
        wr3 = []
        for k in range(BPC):
            ps = aggregate(k, tab2, g_own, cc2, "agg2")
            q = sp_.tile([128, HID], f32, tag="q")
            nc.scalar.activation(out=q[:], in_=ps[:],
                                 func=mybir.ActivationFunctionType.Copy,
                                 scale=dinv[:, k:k + 1])
            qt_ps = pz.tile([128, HID], f32, tag="qt")
            nc.tensor.transpose(out=qt_ps[:], in_=q[:], identity=ident[:])
            qt = sp_.tile([128, HID], f32, tag="qt_sb")
            nc.vector.tensor_copy(out=qt[:], in_=qt_ps[:])
            zps = pz.tile([128, OUT], f32, tag="z")
            nc.tensor.matmul(out=zps[:], lhsT=qt[:], rhs=W2_sb[:], start=True, stop=True)
            z = sp_.tile([128, OUT], f32, tag="zsb")
            nc.vector.tensor_add(out=z[:], in0=zps[:], in1=bias2[:])
            wr3.append(nc.sync.dma_start(out=ag3_in[k * 128:(k + 1) * 128, :], in_=z[:]))

        cc3 = nc.gpsimd.collective_compute(
            "AllGather", mybir.AluOpType.bypass,
            replica_groups=[list(range(C))],
            ins=[ag3_in.ap().opt()], outs=[ztab.ap().opt()],
        )
        for w in wr3:
            add_dep_helper(cc3.ins, w.ins, reason="AG3 after z writes")

        # ---------------- Phase D: decode ----------------
        dec_chunksL = prep["dec_chunks"]
        dec_base = (offA[-1] + offB[-1]) * 8
        ncols = sum(ch for (_, _, ch, _, _) in dec_chunksL)
        out_sb = own.tile([128, ncols], f32)

        acoffs = []
        co = dec_base
        for (g, s, ch, ha, hb) in dec_chunksL:
            acoffs.append(co)
            co += ch * 8
        bcoffs = []
        for (g, s, ch, ha, hb) in dec_chunksL:
            bcoffs.append(co)
            co += ch * 8

        col = 0
        for i, (g, s, ch, ha, hb) in enumerate(dec_chunksL):
            za = dgp.tile([128, ch, OUT], f32, tag="za")
            ga = nc.gpsimd.dma_gather(
                za[:], ztab[(BBASE if ha else 0):, :], idx_sb[:, acoffs[i]:acoffs[i] + ch * 8],
                ch * 128, kreg[ch * 128], OUT, single_packet=False)
            add_dep_helper(ga.ins, cc3.ins, reason="decode a after AG3")
            zb = dgp.tile([128, ch, OUT], f32, tag="zb")
            gb = nc.gpsimd.dma_gather(
                zb[:], ztab[(BBASE if hb else 0):, :], idx_sb[:, bcoffs[i]:bcoffs[i] + ch * 8],
                ch * 128, kreg[ch * 128], OUT, single_packet=False)
            add_dep_helper(gb.ins, cc3.ins, reason="decode b after AG3")
            prod = sp_.tile([128, ch * OUT], f32, tag="prod")
            nc.vector.tensor_mul(out=prod[:].rearrange("p (c o) -> p c o", o=OUT),
                                 in0=za[:], in1=zb[:])
            nc.vector.reduce_sum(out=out_sb[:, col:col + ch],
                                 in_=prod[:].rearrange("p (c o) -> p c o", o=OUT),
                                 axis=mybir.AxisListType.X)
            col += ch
        nc.sync.dma_start(out=out_d[:, :], in_=out_sb[:])

    lower_extended_insts(nc)
    _fix_sync_waits(nc)
    return nc


def kernel(x, W1, b1, W2, b2, edge_index, edge_label_index):
    x = np.asarray(x, dtype=np.float32)
    W1 = np.asarray(W1, dtype=np.float32)
    b1 = np.asarray(b1, dtype=np.float32)
    W2 = np.asarray(W2, dtype=np.float32)
    b2 = np.asarray(b2, dtype=np.float32)
    prep = _prepare(np.asarray(edge_index), np.asarray(edge_label_index))
    nc = _build(prep)

    xp = np.zeros((NP, IN), dtype=np.float32)
    rm = prep["real_mask"]
    xp[rm] = x[prep["final_perm"][rm]]

    in_maps = []
    for c in range(C):
        in_maps.append({
            "xT": np.ascontiguousarray(xp[c * NPC:(c + 1) * NPC].T),
            "W1": W1, "b1": b1.reshape(1, HID),
            "W2": W2, "b2": b2.reshape(1, OUT),
            "degp": prep["degp_core"][c],
            "idx16": prep["idx16"][c],
        })
    res = run_bass_kernel_spmd(nc, in_maps, core_ids=list(range(C)))

    out = np.empty(L, dtype=np.float32)
    for c in range(C):
        o = res.results[c]["out"]          # [128, ncols]
        flat = o.T.reshape(-1)             # position i = (col i//128... careful
        # position i in gather order maps to (p=i%128, chunkcol=i//128)
        ids = prep["dec_sortidx"][c]
        vals = o[np.arange(len(ids)) % 128, np.arange(len(ids)) // 128]
        m = ids >= 0
        out[ids[m]] = vals[m]
    return out


# revision 6
# speedup vs baseline: 1.8192x; 1.8192x over previous
"""GCN (2-layer) + edge-dot decode on 8 TRN2 NeuronCores.

Math (per GCN layer, with dinv = rsqrt(indeg+1)):
    out[v] = dinv[v] * ( sum_{e: dst=v} hs[src_e] + hs[v] ) + b,  hs = dinv (.) (x @ W)
so no per-edge norm values are needed anywhere; all scaling is per-node.

Layer 2 is commuted past W2:  z = (dinv (.) (A_hat g)) @ W2 + b2,  g = dinv (.) relu(out1).

Device layout: nodes permuted (degree-sorted, core-striped so each core owns a
contiguous 6272-row slice). Aggregation via dma_gather row-gathers + TensorE
identity-matmul accumulation into PSUM; hs/g/z tables AllGathered between
phases. dma_gather indices are SIGNED int16, so every gather uses a frame
centered at row 32768 (idx = row - 32768 covers the whole 50176-row table);
the ucode drops a trailing run of negative indices, so each gather appends one
all-positive pad round pointing at a zero (pad-node) row.
"""

import sys
import numpy as np
from contextlib import ExitStack

sys.path.insert(0, "/opt/trn_rl_repo")

import concourse.bass as bass
import concourse.mybir as mybir
from concourse.bass_utils import run_bass_kernel_spmd
from concourse.tile import TileContext, add_dep_helper
from concourse.masks import make_identity
from concourse.library_config import mlp
from concourse.library_overlay import lower_extended_insts

N, E, L = 50000, 800000, 200000
IN, HID, OUT = 256, 128, 64
C = 8                      # cores
NP = 50176                 # padded node count = 392 blocks of 128
NPC = NP // C              # 6272 nodes per core
BPC = NPC // 128           # 49 blocks per core
FBASE = 32768              # gather frame base row (signed int16 centered)
PADIDX = NP - 1 - FBASE    # pad index -> row 50175 (a zero pad-node row), > 0
CH_MAX = 24                # max rounds per gather chunk (excl. appended pad round)
DEC_CHUNK = 16             # decode chunk rounds

CUSTOM_ISA_OPCODES = {"DMAGatherAnt", "DMAScatterAddAnt"}


def _fix_sync_waits(nc):
    """This container's walrus accepts at most one sync-wait per instruction
    and none on custom ISA ucode ops; hoist extras onto preceding drains."""
    f = nc.m.functions[0]
    for b in f.blocks:
        insts = b.instructions
        i = 0
        while i < len(insts):
            ins = insts[i]
            si = ins.sync_info
            nw = len(si.on_wait) if (si is not None and si.on_wait is not None) else 0
            keep = 0 if str(ins.opcode) in CUSTOM_ISA_OPCODES else 1
            if nw > keep:
                waits = list(si.on_wait)
                hoist, keepw = waits[: nw - keep], waits[nw - keep:]
                for j, w in enumerate(hoist):
                    d = mybir.InstDrain(name=f"{ins.name}-wsplit{j}")
                    d.engine = ins.engine
                    d.sync_info = mybir.SyncInfo(on_wait=[w], on_update=[])
                    insts.insert(i + j, d)
                si.on_wait = keepw
                i += len(hoist)
            i += 1


def _sortedpos(p):
    """final position -> position in the degree-sorted sequence."""
    core = p // NPC
    k = (p % NPC) // 128
    lane = p % 128
    return 128 * (8 * k + core) + lane


def _wrap_idx(flat):
    """[n] int16 -> [128, n//16] wrapped in 16 partitions, replicated x8."""
    n = flat.shape[0]
    arr = np.empty((16, n // 16), dtype=np.int16)
    arr[:, :] = flat.reshape(n // 16, 16).T
    return np.tile(arr, (8, 1))


def _prepare(edge_index, edge_label_index):
    src = np.asarray(edge_index[0], dtype=np.int64)
    dst = np.asarray(edge_index[1], dtype=np.int64)
    la = np.asarray(edge_label_index[0], dtype=np.int64)
    lb = np.asarray(edge_label_index[1], dtype=np.int64)

    deg = np.bincount(dst, minlength=N).astype(np.int64)

    # permutation: degree-sorted, core-striped; 176 zero pad nodes at the tail
    sorted_real = np.argsort(-deg, kind="stable")
    seq = np.full(NP, -1, dtype=np.int64)
    seq[:N] = sorted_real
    final_perm = seq[_sortedpos(np.arange(NP))]   # final position -> orig (-1 pad)
    real_mask = final_perm >= 0
    invpos = np.full(N, -1, dtype=np.int64)
    invpos[final_perm[real_mask]] = np.nonzero(real_mask)[0]
    assert final_perm[NP - 1] == -1

    ps = invpos[src]
    pd = invpos[dst]

    # per-node in-edge ranks (dst-major)
    order = np.argsort(pd, kind="stable")
    pd_s = pd[order]
    ps_s = ps[order]
    newgrp = np.empty(E, dtype=bool)
    newgrp[0] = True
    newgrp[1:] = pd_s[1:] != pd_s[:-1]
    gidx = np.nonzero(newgrp)[0]
    rank = np.arange(E) - gidx[np.cumsum(newgrp) - 1]

    lane = pd_s % 128
    core = pd_s // NPC
    slot = (pd_s % NPC) // 128

    nblocks = NP // 128
    KB = np.zeros(nblocks, dtype=np.int64)
    np.maximum.at(KB, pd_s // 128, rank + 1)
    Khat = np.zeros(BPC, dtype=np.int64)
    for k in range(BPC):
        Khat[k] = int(KB[[c * BPC + k for c in range(C)]].max())
    off = np.zeros(BPC + 1, dtype=np.int64)
    off[1:] = np.cumsum(Khat)

    idxT = np.full((C, off[-1], 128), PADIDX, dtype=np.int16)
    idxT[core, off[slot] + rank, lane] = (ps_s - FBASE).astype(np.int16)

    # chunk schedule per block: rounds split into chunks of <= CH_MAX, each
    # gather = chunk rounds + 1 appended all-pad round (trailing positivity)
    chunks = []   # (k, r0, ch)
    for k in range(BPC):
        r = 0
        while r < int(Khat[k]):
            ch = min(CH_MAX, int(Khat[k]) - r)
            chunks.append((k, r, ch))
            r += ch

    # degp per core [128, BPC]
    degp = np.full(NP, 1e30, dtype=np.float32)
    degp[real_mask] = (deg[final_perm[real_mask]] + 1).astype(np.float32)
    degp_core = degp.reshape(C, BPC, 128).transpose(0, 2, 1).copy()

    # decode tables: natural label order per core, chunks of DEC_CHUNK rounds
    pa = invpos[la]
    pb = invpos[lb]
    LPC = L // C
    LROUNDS = (LPC + 127) // 128
    dec_chunks = []
    r = 0
    while r < LROUNDS:
        ch = min(DEC_CHUNK, LROUNDS - r)
        dec_chunks.append((r, ch))
        r += ch

    padrow = np.full(128, PADIDX, dtype=np.int16)
    idx16 = []
    for c in range(C):
        parts = []
        for (k, r0, ch) in chunks:
            flat = idxT[c, off[k] + r0: off[k] + r0 + ch].reshape(-1)
            parts.append(_wrap_idx(np.concatenate([flat, padrow])))
        fa = np.full(LROUNDS * 128, PADIDX, dtype=np.int16)
        fb = np.full(LROUNDS * 128, PADIDX, dtype=np.int16)
        fa[:LPC] = (pa[c * LPC:(c + 1) * LPC] - FBASE).astype(np.int16)
        fb[:LPC] = (pb[c * LPC:(c + 1) * LPC] - FBASE).astype(np.int16)
        for (r0, ch) in dec_chunks:
            parts.append(_wrap_idx(np.concatenate([fa[r0 * 128:(r0 + ch) * 128], padrow])))
        for (r0, ch) in dec_chunks:
            parts.append(_wrap_idx(np.concatenate([fb[r0 * 128:(r0 + ch) * 128], padrow])))
        idx16.append(np.ascontiguousarray(np.concatenate(parts, axis=1)))

    return dict(
        final_perm=final_perm, invpos=invpos, real_mask=real_mask,
        Khat=Khat, off=off, chunks=chunks, dec_chunks=dec_chunks,
        degp_core=degp_core, idx16=idx16,
    )


def _build(prep):
    chunks = prep["chunks"]
    dec_chunks = prep["dec_chunks"]
    TOTW = prep["idx16"][0].shape[1]
    ndec_cols = sum(ch for (_, ch) in dec_chunks)

    nc = bass.Bass(num_devices=C)
    f32 = mybir.dt.float32
    xT_d = nc.dram_tensor("xT", [IN, NPC], f32, kind="ExternalInput")
    W1_d = nc.dram_tensor("W1", [IN, HID], f32, kind="ExternalInput")
    b1_d = nc.dram_tensor("b1", [1, HID], f32, kind="ExternalInput")
    W2_d = nc.dram_tensor("W2", [HID, OUT], f32, kind="ExternalInput")
    b2_d = nc.dram_tensor("b2", [1, OUT], f32, kind="ExternalInput")
    degp_d = nc.dram_tensor("degp", [128, BPC], f32, kind="ExternalInput")
    idx_d = nc.dram_tensor("idx16", [128, TOTW], mybir.dt.int16, kind="ExternalInput")
    out_d = nc.dram_tensor("out", [128, ndec_cols], f32, kind="ExternalOutput")

    ag1_in = nc.dram_tensor("ag1_in", [NPC, HID], f32)
    tab1 = nc.dram_tensor("tab1", [NP, HID], f32)
    ag2_in = nc.dram_tensor("ag2_in", [NPC, HID], f32)
    tab2 = nc.dram_tensor("tab2", [NP, HID], f32)
    ag3_in = nc.dram_tensor("ag3_in", [NPC, OUT], f32)
    ztab = nc.dram_tensor("ztab", [NP, OUT], f32)

    with TileContext(nc) as tc, ExitStack() as ctx:
        const = ctx.enter_context(tc.tile_pool(name="const", bufs=1))
        own = ctx.enter_context(tc.tile_pool(name="own", bufs=1))
        lp = ctx.enter_context(tc.tile_pool(name="lhsT", bufs=4))
        gp = ctx.enter_context(tc.tile_pool(name="gath", bufs=4))
        dgp = ctx.enter_context(tc.tile_pool(name="dgath", bufs=4))
        pp = ctx.enter_context(tc.tile_pool(name="psA", bufs=3, space="PSUM"))
        pz = ctx.enter_context(tc.tile_pool(name="psZ", bufs=2, space="PSUM"))
        sp_ = ctx.enter_context(tc.tile_pool(name="stage", bufs=4))

        ll = nc.gpsimd.load_library(mlp)

        ident = const.tile([128, 128], f32)
        make_identity(nc, ident[:])

        idx_sb = const.tile([128, TOTW], mybir.dt.int16)
        idma = nc.sync.dma_start(out=idx_sb[:], in_=idx_d[:, :])
        add_dep_helper(idma.ins, ll.ins, reason="idx after lib load")

        kvals = sorted({(ch + 1) * 128 for (_, _, ch) in chunks}
                       | {(ch + 1) * 128 for (_, ch) in dec_chunks})
        kreg = {}
        for v in kvals:
            r = ctx.enter_context(nc.gpsimd.register(f"nidx{v}"))
            nc.gpsimd.reg_mov(r, v)
            kreg[v] = r

        W1_sb = []
        for i in range(2):
            w1t = const.tile([128, HID], f32, tag=f"w1_{i}", name=f"w1_{i}")
            nc.sync.dma_start(out=w1t[:], in_=W1_d[i * 128:(i + 1) * 128, :])
            W1_sb.append(w1t)
        W2_sb = const.tile([128, OUT], f32)
        nc.sync.dma_start(out=W2_sb[:], in_=W2_d[:, :])

        ones_row = const.tile([1, 128], f32)
        nc.vector.memset(ones_row[:], 1.0)
        b1_row = const.tile([1, HID], f32)
        nc.sync.dma_start(out=b1_row[:], in_=b1_d[:, :])
        b2_row = const.tile([1, OUT], f32)
        nc.sync.dma_start(out=b2_row[:], in_=b2_d[:, :])
        bias1 = const.tile([128, HID], f32)
        bps = pz.tile([128, HID], f32, tag="qt")
        nc.tensor.matmul(out=bps[:], lhsT=ones_row[:], rhs=b1_row[:], start=True, stop=True)
        nc.scalar.activation(out=bias1[:], in_=bps[:], func=mybir.ActivationFunctionType.Copy)
        bias2 = const.tile([128, OUT], f32)
        bps2 = pz.tile([128, OUT], f32, tag="qt")
        nc.tensor.matmul(out=bps2[:], lhsT=ones_row[:], rhs=b2_row[:], start=True, stop=True)
        nc.scalar.activation(out=bias2[:], in_=bps2[:], func=mybir.ActivationFunctionType.Copy)

        degp_sb = const.tile([128, BPC], f32)
        nc.sync.dma_start(out=degp_sb[:], in_=degp_d[:, :])
        rec = const.tile([128, BPC], f32)
        nc.vector.reciprocal(out=rec[:], in_=degp_sb[:])
        dinv = const.tile([128, BPC], f32)
        nc.scalar.activation(out=dinv[:], in_=rec[:], func=mybir.ActivationFunctionType.Sqrt)

        hs_own = own.tile([128, NPC], f32)
        g_own = own.tile([128, NPC], f32)

        # chunk -> idx column offsets
        blk_chunks = [[] for _ in range(BPC)]
        co = 0
        for (k, r0, ch) in chunks:
            blk_chunks[k].append((co, ch))
            co += (ch + 1) * 8
        dec_acoffs = []
        for (r0, ch) in dec_chunks:
            dec_acoffs.append(co)
            co += (ch + 1) * 8
        dec_bcoffs = []
        for (r0, ch) in dec_chunks:
            dec_bcoffs.append(co)
            co += (ch + 1) * 8
        assert co == TOTW

        # ---------------- Phase A: GEMM1 -> hs1 ----------------
        wr1 = []
        with nc.named_scope("gemm1"):
            for k in range(BPC):
                ps = pp.tile([128, HID], f32, tag="main", name="psg")
                for i in range(2):
                    lt = lp.tile([128, 128], f32, tag="lhsT", name="lt")
                    nc.sync.dma_start(out=lt[:], in_=xT_d[i * 128:(i + 1) * 128, k * 128:(k + 1) * 128])
                    nc.tensor.matmul(out=ps[:], lhsT=lt[:], rhs=W1_sb[i][:],
                                     start=(i == 0), stop=(i == 1))
                nc.scalar.activation(out=hs_own[:, k * 128:(k + 1) * 128], in_=ps[:],
                                     func=mybir.ActivationFunctionType.Copy,
                                     scale=dinv[:, k:k + 1])
                wr1.append(nc.sync.dma_start(out=ag1_in[k * 128:(k + 1) * 128, :],
                                             in_=hs_own[:, k * 128:(k + 1) * 128]))

        with nc.named_scope("ag1"):
            cc1 = nc.gpsimd.collective_compute(
                "AllGather", mybir.AluOpType.bypass,
                replica_groups=[list(range(C))],
                ins=[ag1_in.ap().opt()], outs=[tab1.ap().opt()],
            )
            for w in wr1:
                add_dep_helper(cc1.ins, w.ins, reason="AG1 after hs writes")

        def aggregate(k, tab, own_tile, cc):
            ps = pp.tile([128, HID], f32, tag="main", name="psagg")
            first = True
            for (coff, ch) in blk_chunks[k]:
                gt = gp.tile([128, ch + 1, HID], f32, tag="gt", name="gt")
                gi = nc.gpsimd.dma_gather(
                    gt[:], tab[FBASE:, :], idx_sb[:, coff:coff + (ch + 1) * 8],
                    (ch + 1) * 128, kreg[(ch + 1) * 128], HID, single_packet=False)
                add_dep_helper(gi.ins, cc.ins, reason="gather after AG")
                for r in range(ch):
                    nc.tensor.matmul(out=ps[:], lhsT=ident[:], rhs=gt[:, r, :],
                                     start=first, stop=False)
                    first = False
            nc.tensor.matmul(out=ps[:], lhsT=ident[:],
                             rhs=own_tile[:, k * 128:(k + 1) * 128],
                             start=first, stop=True)
            return ps

        # ---------------- Phase B: layer-1 aggregation -> g ----------------
        wr2 = []
        with nc.named_scope("agg1"):
            for k in range(BPC):
                ps = aggregate(k, tab1, hs_own, cc1)
                t1 = sp_.tile([128, HID], f32, tag="t1", name="t1")
                nc.scalar.activation(out=t1[:], in_=ps[:],
                                     func=mybir.ActivationFunctionType.Copy,
                                     scale=dinv[:, k:k + 1])
                t2 = sp_.tile([128, HID], f32, tag="t2", name="t2")
                nc.vector.tensor_add(out=t2[:], in0=t1[:], in1=bias1[:])
                nc.scalar.activation(out=g_own[:, k * 128:(k + 1) * 128], in_=t2[:],
                                     func=mybir.ActivationFunctionType.Relu,
                                     scale=dinv[:, k:k + 1])
                wr2.append(nc.sync.dma_start(out=ag2_in[k * 128:(k + 1) * 128, :],
                                             in_=g_own[:, k * 128:(k + 1) * 128]))

        with nc.named_scope("ag2"):
            cc2 = nc.gpsimd.collective_compute(
                "AllGather", mybir.AluOpType.bypass,
                replica_groups=[list(range(C))],
                ins=[ag2_in.ap().opt()], outs=[tab2.ap().opt()],
            )
            for w in wr2:
                add_dep_helper(cc2.ins, w.ins, reason="AG2 after g writes")

        # ---------------- Phase C: layer-2 aggregation + GEMM2 -> z ----------------
        wr3 = []
        with nc.named_scope("agg2"):
            for k in range(BPC):
                ps = aggregate(k, tab2, g_own, cc2)
                q = sp_.tile([128, HID], f32, tag="q", name="q")
                nc.scalar.activation(out=q[:], in_=ps[:],
                                     func=mybir.ActivationFunctionType.Copy,
                                     scale=dinv[:, k:k + 1])
                qt_ps = pz.tile([128, HID], f32, tag="qt", name="qtps")
                nc.tensor.transpose(out=qt_ps[:], in_=q[:], identity=ident[:])
                qt = sp_.tile([128, HID], f32, tag="qt_sb", name="qtsb")
                nc.vector.tensor_copy(out=qt[:], in_=qt_ps[:])
                zps = pz.tile([128, OUT], f32, tag="z", name="zps")
                nc.tensor.matmul(out=zps[:], lhsT=qt[:], rhs=W2_sb[:], start=True, stop=True)
                z = sp_.tile([128, OUT], f32, tag="zsb", name="zsb")
                nc.vector.tensor_add(out=z[:], in0=zps[:], in1=bias2[:])
                wr3.append(nc.sync.dma_start(out=ag3_in[k * 128:(k + 1) * 128, :], in_=z[:]))

        with nc.named_scope("ag3"):
            cc3 = nc.gpsimd.collective_compute(
                "AllGather", mybir.AluOpType.bypass,
                replica_groups=[list(range(C))],
                ins=[ag3_in.ap().opt()], outs=[ztab.ap().opt()],
            )
            for w in wr3:
                add_dep_helper(cc3.ins, w.ins, reason="AG3 after z writes")

        # ---------------- Phase D: decode ----------------
        with nc.named_scope("decode"):
            out_sb = own.tile([128, ndec_cols], f32)
            col = 0
            for i, (r0, ch) in enumerate(dec_chunks):
                za = dgp.tile([128, ch + 1, OUT], f32, tag="za", name="za")
                ga = nc.gpsimd.dma_gather(
                    za[:], ztab[FBASE:, :], idx_sb[:, dec_acoffs[i]:dec_acoffs[i] + (ch + 1) * 8],
                    (ch + 1) * 128, kreg[(ch + 1) * 128], OUT, single_packet=False)
                add_dep_helper(ga.ins, cc3.ins, reason="decode a after AG3")
                zb = dgp.tile([128, ch + 1, OUT], f32, tag="zb", name="zb")
                gb = nc.gpsimd.dma_gather(
                    zb[:], ztab[FBASE:, :], idx_sb[:, dec_bcoffs[i]:dec_bcoffs[i] + (ch + 1) * 8],
                    (ch + 1) * 128, kreg[(ch + 1) * 128], OUT, single_packet=False)
                add_dep_helper(gb.ins, cc3.ins, reason="decode b after AG3")
                prod = sp_.tile([128, ch * OUT], f32, tag="prod", name="prod")
                nc.vector.tensor_mul(out=prod[:].rearrange("p (c o) -> p c o", o=OUT),
                                     in0=za[:, :ch, :], in1=zb[:, :ch, :])
                nc.vector.reduce_sum(out=out_sb[:, col:col + ch],
                                     in_=prod[:].rearrange("p (c o) -> p c o", o=OUT),
                                     axis=mybir.AxisListType.X)
                col += ch
            nc.sync.dma_start(out=out_d[:, :], in_=out_sb[:])

    lower_extended_insts(nc)
    _fix_sync_waits(nc)
    return nc


def kernel(x, W1, b1, W2, b2, edge_index, edge_label_index):
    x = np.asarray(x, dtype=np.float32)
    W1 = np.asarray(W1, dtype=np.float32)
    b1 = np.asarray(b1, dtype=np.float32)
    W2 = np.asarray(W2, dtype=np.float32)
    b2 = np.asarray(b2, dtype=np.float32)
    prep = _prepare(np.asarray(edge_index), np.asarray(edge_label_index))
    nc = _build(prep)

    xp = np.zeros((NP, IN), dtype=np.float32)
    rm = prep["real_mask"]
    xp[rm] = x[prep["final_perm"][rm]]

    in_maps = []
    for c in range(C):
        in_maps.append({
            "xT": np.ascontiguousarray(xp[c * NPC:(c + 1) * NPC].T),
            "W1": W1, "b1": b1.reshape(1, HID),
            "W2": W2, "b2": b2.reshape(1, OUT),
            "degp": prep["degp_core"][c],
            "idx16": prep["idx16"][c],
        })
    res = run_bass_kernel_spmd(nc, in_maps, core_ids=list(range(C)))

    LPC = L // C
    out = np.empty(L, dtype=np.float32)
    for c in range(C):
        o = res.results[c]["out"]          # [128, ncols]; label j at (j%128, j//128)
        j = np.arange(LPC)
        out[c * LPC:(c + 1) * LPC] = o[j % 128, j // 128]
    return out


# revision 8
# speedup vs baseline: 1.8700x; 1.0279x over previous
"""GCN (2-layer) + edge-dot decode on 8 TRN2 NeuronCores.

Math (per GCN layer, with dinv = rsqrt(indeg+1)):
    out[v] = dinv[v] * ( sum_{e: dst=v} hs[src_e] + hs[v] ) + b,  hs = dinv (.) (x @ W)
so no per-edge norm values are needed anywhere; all scaling is per-node.

Layer 2 is commuted past W2:  z = (dinv (.) (A_hat g)) @ W2 + b2,  g = dinv (.) relu(out1).

Device layout: nodes permuted (degree-sorted, core-striped so each core owns a
contiguous 6272-row slice). Aggregation via dma_gather row-gathers + TensorE
identity-matmul accumulation into PSUM; hs/g/z tables AllGathered between
phases. dma_gather indices are SIGNED int16, so every gather uses a frame
centered at row 32768 (idx = row - 32768 covers the whole 50176-row table);
the ucode drops a trailing run of negative indices, so each gather appends one
all-positive pad round pointing at a zero (pad-node) row.
"""

import sys
import numpy as np
from contextlib import ExitStack

sys.path.insert(0, "/opt/trn_rl_repo")

import concourse.bass as bass
import concourse.mybir as mybir
from concourse.bass_utils import run_bass_kernel_spmd
from concourse.tile import TileContext, add_dep_helper
from concourse.masks import make_identity
from concourse.library_config import mlp
from concourse.library_overlay import lower_extended_insts

N, E, L = 50000, 800000, 200000
IN, HID, OUT = 256, 128, 64
C = 8                      # cores
NP = 50176                 # padded node count = 392 blocks of 128
NPC = NP // C              # 6272 nodes per core
BPC = NPC // 128           # 49 blocks per core
FBASE = 32768              # gather frame base row (signed int16 centered)
PADIDX = NP - 1 - FBASE    # pad index -> row 50175 (a zero pad-node row), > 0
CH_MAX = 24                # max rounds per gather chunk (excl. appended pad round)
DEC_CHUNK = 16             # decode chunk rounds

CUSTOM_ISA_OPCODES = {"DMAGatherAnt", "DMAScatterAddAnt"}


def _fix_sync_waits(nc):
    """This container's walrus accepts at most one sync-wait per instruction
    and none on custom ISA ucode ops; hoist extras onto preceding drains."""
    f = nc.m.functions[0]
    for b in f.blocks:
        insts = b.instructions
        i = 0
        while i < len(insts):
            ins = insts[i]
            si = ins.sync_info
            nw = len(si.on_wait) if (si is not None and si.on_wait is not None) else 0
            keep = 0 if str(ins.opcode) in CUSTOM_ISA_OPCODES else 1
            if nw > keep:
                waits = list(si.on_wait)
                hoist, keepw = waits[: nw - keep], waits[nw - keep:]
                for j, w in enumerate(hoist):
                    d = mybir.InstDrain(name=f"{ins.name}-wsplit{j}")
                    d.engine = ins.engine
                    d.sync_info = mybir.SyncInfo(on_wait=[w], on_update=[])
                    insts.insert(i + j, d)
                si.on_wait = keepw
                i += len(hoist)
            i += 1


def _sortedpos(p):
    """final position -> position in the degree-sorted sequence."""
    core = p // NPC
    k = (p % NPC) // 128
    lane = p % 128
    return 128 * (8 * k + core) + lane


def _wrap_idx(flat):
    """[n] int16 -> [128, n//16] wrapped in 16 partitions, replicated x8."""
    n = flat.shape[0]
    arr = np.empty((16, n // 16), dtype=np.int16)
    arr[:, :] = flat.reshape(n // 16, 16).T
    return np.tile(arr, (8, 1))


def _prepare(edge_index, edge_label_index):
    src = np.asarray(edge_index[0], dtype=np.int64)
    dst = np.asarray(edge_index[1], dtype=np.int64)
    la = np.asarray(edge_label_index[0], dtype=np.int64)
    lb = np.asarray(edge_label_index[1], dtype=np.int64)

    deg = np.bincount(dst, minlength=N).astype(np.int64)

    # permutation: degree-sorted, core-striped; 176 zero pad nodes at the tail
    sorted_real = np.argsort(-deg, kind="stable")
    seq = np.full(NP, -1, dtype=np.int64)
    seq[:N] = sorted_real
    final_perm = seq[_sortedpos(np.arange(NP))]   # final position -> orig (-1 pad)
    real_mask = final_perm >= 0
    invpos = np.full(N, -1, dtype=np.int64)
    invpos[final_perm[real_mask]] = np.nonzero(real_mask)[0]
    assert final_perm[NP - 1] == -1

    ps = invpos[src]
    pd = invpos[dst]

    # per-node in-edge ranks (dst-major)
    order = np.argsort(pd, kind="stable")
    pd_s = pd[order]
    ps_s = ps[order]
    newgrp = np.empty(E, dtype=bool)
    newgrp[0] = True
    newgrp[1:] = pd_s[1:] != pd_s[:-1]
    gidx = np.nonzero(newgrp)[0]
    rank = np.arange(E) - gidx[np.cumsum(newgrp) - 1]

    lane = pd_s % 128
    core = pd_s // NPC
    slot = (pd_s % NPC) // 128

    nblocks = NP // 128
    KB = np.zeros(nblocks, dtype=np.int64)
    np.maximum.at(KB, pd_s // 128, rank + 1)
    Khat = np.zeros(BPC, dtype=np.int64)
    for k in range(BPC):
        Khat[k] = int(KB[[c * BPC + k for c in range(C)]].max())
    off = np.zeros(BPC + 1, dtype=np.int64)
    off[1:] = np.cumsum(Khat)

    idxT = np.full((C, off[-1], 128), PADIDX, dtype=np.int16)
    idxT[core, off[slot] + rank, lane] = (ps_s - FBASE).astype(np.int16)

    # chunk schedule per block: rounds split into chunks of <= CH_MAX, each
    # gather = chunk rounds + 1 appended all-pad round (trailing positivity)
    chunks = []   # (k, r0, ch)
    for k in range(BPC):
        r = 0
        while r < int(Khat[k]):
            ch = min(CH_MAX, int(Khat[k]) - r)
            chunks.append((k, r, ch))
            r += ch

    # degp per core [128, BPC]
    degp = np.full(NP, 1e30, dtype=np.float32)
    degp[real_mask] = (deg[final_perm[real_mask]] + 1).astype(np.float32)
    degp_core = degp.reshape(C, BPC, 128).transpose(0, 2, 1).copy()

    # decode tables: natural label order per core, chunks of DEC_CHUNK rounds
    pa = invpos[la]
    pb = invpos[lb]
    LPC = L // C
    LROUNDS = (LPC + 127) // 128
    dec_chunks = []
    r = 0
    while r < LROUNDS:
        ch = min(DEC_CHUNK, LROUNDS - r)
        dec_chunks.append((r, ch))
        r += ch

    padrow = np.full(128, PADIDX, dtype=np.int16)
    idx16 = []
    for c in range(C):
        parts = []
        for (k, r0, ch) in chunks:
            flat = idxT[c, off[k] + r0: off[k] + r0 + ch].reshape(-1)
            parts.append(_wrap_idx(np.concatenate([flat, padrow])))
        fa = np.full(LROUNDS * 128, PADIDX, dtype=np.int16)
        fb = np.full(LROUNDS * 128, PADIDX, dtype=np.int16)
        fa[:LPC] = (pa[c * LPC:(c + 1) * LPC] - FBASE).astype(np.int16)
        fb[:LPC] = (pb[c * LPC:(c + 1) * LPC] - FBASE).astype(np.int16)
        for (r0, ch) in dec_chunks:
            parts.append(_wrap_idx(np.concatenate([fa[r0 * 128:(r0 + ch) * 128], padrow])))
        for (r0, ch) in dec_chunks:
            parts.append(_wrap_idx(np.concatenate([fb[r0 * 128:(r0 + ch) * 128], padrow])))
        idx16.append(np.ascontiguousarray(np.concatenate(parts, axis=1)))

    return dict(
        final_perm=final_perm, invpos=invpos, real_mask=real_mask,
        Khat=Khat, off=off, chunks=chunks, dec_chunks=dec_chunks,
        degp_core=degp_core, idx16=idx16,
    )


def _build(prep):
    chunks = prep["chunks"]
    dec_chunks = prep["dec_chunks"]
    TOTW = prep["idx16"][0].shape[1]
    ndec_cols = sum(ch for (_, ch) in dec_chunks)

    nc = bass.Bass(num_devices=C, dynamic_dma_scratch_size=32768, num_swdge_queues=2)
    f32 = mybir.dt.float32
    xT_d = nc.dram_tensor("xT", [IN, NPC], f32, kind="ExternalInput")
    W1_d = nc.dram_tensor("W1", [IN, HID], f32, kind="ExternalInput")
    b1_d = nc.dram_tensor("b1", [1, HID], f32, kind="ExternalInput")
    W2_d = nc.dram_tensor("W2", [HID, OUT], f32, kind="ExternalInput")
    b2_d = nc.dram_tensor("b2", [1, OUT], f32, kind="ExternalInput")
    degp_d = nc.dram_tensor("degp", [128, BPC], f32, kind="ExternalInput")
    idx_d = nc.dram_tensor("idx16", [128, TOTW], mybir.dt.int16, kind="ExternalInput")
    out_d = nc.dram_tensor("out", [128, ndec_cols], f32, kind="ExternalOutput")

    ag1_in = nc.dram_tensor("ag1_in", [NPC, HID], f32)
    tab1 = nc.dram_tensor("tab1", [NP, HID], f32)
    ag2_in = nc.dram_tensor("ag2_in", [NPC, HID], f32)
    tab2 = nc.dram_tensor("tab2", [NP, HID], f32)
    ag3_in = nc.dram_tensor("ag3_in", [NPC, OUT], f32)
    ztab = nc.dram_tensor("ztab", [NP, OUT], f32)

    with TileContext(nc) as tc, ExitStack() as ctx:
        const = ctx.enter_context(tc.tile_pool(name="const", bufs=1))
        own = ctx.enter_context(tc.tile_pool(name="own", bufs=1))
        lp = ctx.enter_context(tc.tile_pool(name="lhsT", bufs=4))
        gp = ctx.enter_context(tc.tile_pool(name="gath", bufs=4))
        dgp = ctx.enter_context(tc.tile_pool(name="dgath", bufs=3))
        pp = ctx.enter_context(tc.tile_pool(name="psA", bufs=3, space="PSUM"))
        pz = ctx.enter_context(tc.tile_pool(name="psZ", bufs=2, space="PSUM"))
        sp_ = ctx.enter_context(tc.tile_pool(name="stage", bufs=4))

        ll = nc.gpsimd.load_library(mlp)

        ident = const.tile([128, 128], f32)
        make_identity(nc, ident[:])

        idx_sb = const.tile([128, TOTW], mybir.dt.int16)
        idma = nc.sync.dma_start(out=idx_sb[:], in_=idx_d[:, :])
        add_dep_helper(idma.ins, ll.ins, reason="idx after lib load")

        kvals = sorted({(ch + 1) * 128 for (_, _, ch) in chunks}
                       | {(ch + 1) * 128 for (_, ch) in dec_chunks})
        kreg = {}
        for v in kvals:
            r = ctx.enter_context(nc.gpsimd.register(f"nidx{v}"))
            nc.gpsimd.reg_mov(r, v)
            kreg[v] = r

        W1_sb = []
        for i in range(2):
            w1t = const.tile([128, HID], f32, tag=f"w1_{i}", name=f"w1_{i}")
            nc.sync.dma_start(out=w1t[:], in_=W1_d[i * 128:(i + 1) * 128, :])
            W1_sb.append(w1t)
        W2_sb = const.tile([128, OUT], f32)
        nc.sync.dma_start(out=W2_sb[:], in_=W2_d[:, :])

        ones_row = const.tile([1, 128], f32)
        nc.vector.memset(ones_row[:], 1.0)
        b1_row = const.tile([1, HID], f32)
        nc.sync.dma_start(out=b1_row[:], in_=b1_d[:, :])
        b2_row = const.tile([1, OUT], f32)
        nc.sync.dma_start(out=b2_row[:], in_=b2_d[:, :])
        bias1 = const.tile([128, HID], f32)
        bps = pz.tile([128, HID], f32, tag="qt")
        nc.tensor.matmul(out=bps[:], lhsT=ones_row[:], rhs=b1_row[:], start=True, stop=True)
        nc.scalar.activation(out=bias1[:], in_=bps[:], func=mybir.ActivationFunctionType.Copy)
        bias2 = const.tile([128, OUT], f32)
        bps2 = pz.tile([128, OUT], f32, tag="qt")
        nc.tensor.matmul(out=bps2[:], lhsT=ones_row[:], rhs=b2_row[:], start=True, stop=True)
        nc.scalar.activation(out=bias2[:], in_=bps2[:], func=mybir.ActivationFunctionType.Copy)

        degp_sb = const.tile([128, BPC], f32)
        nc.sync.dma_start(out=degp_sb[:], in_=degp_d[:, :])
        rec = const.tile([128, BPC], f32)
        nc.vector.reciprocal(out=rec[:], in_=degp_sb[:])
        dinv = const.tile([128, BPC], f32)
        nc.scalar.activation(out=dinv[:], in_=rec[:], func=mybir.ActivationFunctionType.Sqrt)

        hs_own = own.tile([128, NPC], f32)
        g_own = own.tile([128, NPC], f32)

        # chunk -> idx column offsets
        blk_chunks = [[] for _ in range(BPC)]
        co = 0
        for (k, r0, ch) in chunks:
            blk_chunks[k].append((co, ch))
            co += (ch + 1) * 8
        dec_acoffs = []
        for (r0, ch) in dec_chunks:
            dec_acoffs.append(co)
            co += (ch + 1) * 8
        dec_bcoffs = []
        for (r0, ch) in dec_chunks:
            dec_bcoffs.append(co)
            co += (ch + 1) * 8
        assert co == TOTW

        # ---------------- Phase A: GEMM1 -> hs1 ----------------
        wr1 = []
        with nc.named_scope("gemm1"):
            for k in range(BPC):
                ps = pp.tile([128, HID], f32, tag="main", name="psg")
                for i in range(2):
                    lt = lp.tile([128, 128], f32, tag="lhsT", name="lt")
                    nc.sync.dma_start(out=lt[:], in_=xT_d[i * 128:(i + 1) * 128, k * 128:(k + 1) * 128])
                    nc.tensor.matmul(out=ps[:], lhsT=lt[:], rhs=W1_sb[i][:],
                                     start=(i == 0), stop=(i == 1))
                nc.scalar.activation(out=hs_own[:, k * 128:(k + 1) * 128], in_=ps[:],
                                     func=mybir.ActivationFunctionType.Copy,
                                     scale=dinv[:, k:k + 1])
                wr1.append(nc.sync.dma_start(out=ag1_in[k * 128:(k + 1) * 128, :],
                                             in_=hs_own[:, k * 128:(k + 1) * 128]))

        with nc.named_scope("ag1"):
            cc1 = nc.gpsimd.collective_compute(
                "AllGather", mybir.AluOpType.bypass,
                replica_groups=[list(range(C))],
                ins=[ag1_in.ap().opt()], outs=[tab1.ap().opt()],
            )
            for w in wr1:
                add_dep_helper(cc1.ins, w.ins, reason="AG1 after hs writes")

        qctr = [0]

        def aggregate(k, tab, own_tile, cc):
            ps = pp.tile([128, HID], f32, tag="main", name="psagg")
            first = True
            for (coff, ch) in blk_chunks[k]:
                gt = gp.tile([128, ch + 1, HID], f32, tag="gt", name="gt")
                qctr[0] ^= 1
                gi = nc.gpsimd.dma_gather(
                    gt[:], tab[FBASE:, :], idx_sb[:, coff:coff + (ch + 1) * 8],
                    (ch + 1) * 128, kreg[(ch + 1) * 128], HID, single_packet=False,
                    queue_num=qctr[0])
                add_dep_helper(gi.ins, cc.ins, reason="gather after AG")
                for r in range(ch):
                    nc.tensor.matmul(out=ps[:], lhsT=ident[:], rhs=gt[:, r, :],
                                     start=first, stop=False)
                    first = False
            nc.tensor.matmul(out=ps[:], lhsT=ident[:],
                             rhs=own_tile[:, k * 128:(k + 1) * 128],
                             start=first, stop=True)
            return ps

        # ---------------- Phase B: layer-1 aggregation -> g ----------------
        wr2 = []
        with nc.named_scope("agg1"):
            for k in range(BPC):
                ps = aggregate(k, tab1, hs_own, cc1)
                t1 = sp_.tile([128, HID], f32, tag="t1", name="t1")
                nc.scalar.activation(out=t1[:], in_=ps[:],
                                     func=mybir.ActivationFunctionType.Copy,
                                     scale=dinv[:, k:k + 1])
                t2 = sp_.tile([128, HID], f32, tag="t2", name="t2")
                nc.vector.tensor_add(out=t2[:], in0=t1[:], in1=bias1[:])
                nc.scalar.activation(out=g_own[:, k * 128:(k + 1) * 128], in_=t2[:],
                                     func=mybir.ActivationFunctionType.Relu,
                                     scale=dinv[:, k:k + 1])
                wr2.append(nc.sync.dma_start(out=ag2_in[k * 128:(k + 1) * 128, :],
                                             in_=g_own[:, k * 128:(k + 1) * 128]))

        with nc.named_scope("ag2"):
            cc2 = nc.gpsimd.collective_compute(
                "AllGather", mybir.AluOpType.bypass,
                replica_groups=[list(range(C))],
                ins=[ag2_in.ap().opt()], outs=[tab2.ap().opt()],
            )
            for w in wr2:
                add_dep_helper(cc2.ins, w.ins, reason="AG2 after g writes")

        # ---------------- Phase C: layer-2 aggregation + GEMM2 -> z ----------------
        wr3 = []
        with nc.named_scope("agg2"):
            for k in range(BPC):
                ps = aggregate(k, tab2, g_own, cc2)
                q = sp_.tile([128, HID], f32, tag="q", name="q")
                nc.scalar.activation(out=q[:], in_=ps[:],
                                     func=mybir.ActivationFunctionType.Copy,
                                     scale=dinv[:, k:k + 1])
                qt_ps = pz.tile([128, HID], f32, tag="qt", name="qtps")
                nc.tensor.transpose(out=qt_ps[:], in_=q[:], identity=ident[:])
                qt = sp_.tile([128, HID], f32, tag="qt_sb", name="qtsb")
                nc.vector.tensor_copy(out=qt[:], in_=qt_ps[:])
                zps = pz.tile([128, OUT], f32, tag="z", name="zps")
                nc.tensor.matmul(out=zps[:], lhsT=qt[:], rhs=W2_sb[:], start=True, stop=True)
                z = sp_.tile([128, OUT], f32, tag="zsb", name="zsb")
                nc.vector.tensor_add(out=z[:], in0=zps[:], in1=bias2[:])
                wr3.append(nc.sync.dma_start(out=ag3_in[k * 128:(k + 1) * 128, :], in_=z[:]))

        with nc.named_scope("ag3"):
            cc3 = nc.gpsimd.collective_compute(
                "AllGather", mybir.AluOpType.bypass,
                replica_groups=[list(range(C))],
                ins=[ag3_in.ap().opt()], outs=[ztab.ap().opt()],
            )
            for w in wr3:
                add_dep_helper(cc3.ins, w.ins, reason="AG3 after z writes")

        # ---------------- Phase D: decode ----------------
        with nc.named_scope("decode"):
            out_sb = own.tile([128, ndec_cols], f32)
            col = 0
            for i, (r0, ch) in enumerate(dec_chunks):
                za = dgp.tile([128, ch + 1, OUT], f32, tag="za", name="za")
                ga = nc.gpsimd.dma_gather(
                    za[:], ztab[FBASE:, :], idx_sb[:, dec_acoffs[i]:dec_acoffs[i] + (ch + 1) * 8],
                    (ch + 1) * 128, kreg[(ch + 1) * 128], OUT, single_packet=False,
                    queue_num=0)
                add_dep_helper(ga.ins, cc3.ins, reason="decode a after AG3")
                zb = dgp.tile([128, ch + 1, OUT], f32, tag="zb", name="zb")
                gb = nc.gpsimd.dma_gather(
                    zb[:], ztab[FBASE:, :], idx_sb[:, dec_bcoffs[i]:dec_bcoffs[i] + (ch + 1) * 8],
                    (ch + 1) * 128, kreg[(ch + 1) * 128], OUT, single_packet=False,
                    queue_num=1)
                add_dep_helper(gb.ins, cc3.ins, reason="decode b after AG3")
                prod = sp_.tile([128, ch * OUT], f32, tag="prod", name="prod")
                nc.vector.tensor_mul(out=prod[:].rearrange("p (c o) -> p c o", o=OUT),
                                     in0=za[:, :ch, :], in1=zb[:, :ch, :])
                nc.vector.reduce_sum(out=out_sb[:, col:col + ch],
                                     in_=prod[:].rearrange("p (c o) -> p c o", o=OUT),
                                     axis=mybir.AxisListType.X)
                col += ch
            nc.sync.dma_start(out=out_d[:, :], in_=out_sb[:])

    lower_extended_insts(nc)
    _fix_sync_waits(nc)
    return nc


def kernel(x, W1, b1, W2, b2, edge_index, edge_label_index):
    x = np.asarray(x, dtype=np.float32)
    W1 = np.asarray(W1, dtype=np.float32)
    b1 = np.asarray(b1, dtype=np.float32)
    W2 = np.asarray(W2, dtype=np.float32)
    b2 = np.asarray(b2, dtype=np.float32)
    prep = _prepare(np.asarray(edge_index), np.asarray(edge_label_index))
    nc = _build(prep)

    xp = np.zeros((NP, IN), dtype=np.float32)
    rm = prep["real_mask"]
    xp[rm] = x[prep["final_perm"][rm]]

    in_maps = []
    for c in range(C):
        in_maps.append({
            "xT": np.ascontiguousarray(xp[c * NPC:(c + 1) * NPC].T),
            "W1": W1, "b1": b1.reshape(1, HID),
            "W2": W2, "b2": b2.reshape(1, OUT),
            "degp": prep["degp_core"][c],
            "idx16": prep["idx16"][c],
        })
    res = run_bass_kernel_spmd(nc, in_maps, core_ids=list(range(C)))

    LPC = L // C
    out = np.empty(L, dtype=np.float32)
    for c in range(C):
        o = res.results[c]["out"]          # [128, ncols]; label j at (j%128, j//128)
        j = np.arange(LPC)
        out[c * LPC:(c + 1) * LPC] = o[j % 128, j // 128]
    return out
